# revision 29
# baseline (speedup 1.0000x reference)
"""AttentionGCNLayer Trainium2 kernel.

Per-sample computation (B=8 samples -> 8 NeuronCores, data-parallel):
  identity = x @ W_it + b_it
  gcn      = relu(adj @ (x @ W_g + b_g))
  h        = LN1(identity + gcn)
  attn     = MHSA(h)  (8 heads, D=32)
  out      = LN2(h + attn)

Design notes (~2.1x faster than the previous kernel, ~117us/core):
  - All layout work on the host: x^T and adj^T uploaded pre-transposed in
    bf16, LN1 gamma/beta folded into W_q/W_k/W_v, broadcast bias tiles
    pre-replicated, shared weights packed into two DMA blobs ordered so
    phase-1 inputs and the first adj^T half land first.
  - Every matmul is bf16 (fp32 runs at 4 cycles/row on the PE; bf16 at 1).
  - Softmax exp of the 8.4M-element score matrix is the dominant
    elementwise cost (a PSUM->SBUF crossing at ~4B/lane/cycle, ~65us of
    engine time).  Each score tile is split column-wise: ScalarE does
    exact exp on cols [0,EA), VectorE does Schraudolph bit-trick exp
    (i16 = trunc(s*SCALE*128/ln2 + 16249); bitcast i16 -> bf16) on
    [EA,1024) -- both engines run concurrently on the same tile.
    End-to-end error of the trick is negligible (3.41e-3 vs 3.41e-3
    exact; tolerance 2e-2).
  - The attention runs as ONE flat software pipeline over all 4
    (token-half x head-group) groups x 8 key-chunks: attn@V/denominator
    matmuls lag the score matmuls by PIPE iterations ACROSS group
    boundaries, each group's softmax normalization fires when its last
    attn matmul issues, and all 4 score matmuls of an iteration are
    emitted back-to-back (4 distinct 32-row PE bands -> one concurrent
    array fill).  The score PSUM pool keeps 3 tile generations so the
    exp engines stay off the PSUM-recycle critical cycle.
  - A short burst of dummy matmuls gated on the xT DMA trips the PE HAM
    activity window so phase 1 / GCN run at 2.4 GHz instead of 1.2.
  - k-projection bias dropped: softmax is invariant to score shifts
    constant along the key axis.
  - LN2 residual (hhat*g1) folded into the output projection as diag(g1)
    matmuls; (beta1+b_o) preloaded into PSUM via a contraction-1 ones
    matmul; LN rsqrt batched on VectorE (quake seed + 2 Newton steps);
    LN2 normalize on ScalarE via Identity(scale=rstd, bias=-mu*rstd).
  - Softmax denominators via ones-matmul (partition-direction sum on
    PE), scoresT layout [key on partition, query on free].
  - Output returned bf16 and cast to fp32 on the host.
"""

import sys

sys.path.insert(0, "/opt/trn_rl_repo")

import numpy as np

import concourse.bass as bass
import concourse.tile as tile
from concourse import bacc, mybir
from concourse.bass_utils import run_bass_kernel_spmd
from concourse.masks import make_identity

F32 = mybir.dt.float32
BF16 = mybir.dt.bfloat16
I16 = mybir.dt.int16
I32 = mybir.dt.int32
AF = mybir.ActivationFunctionType
ALU = mybir.AluOpType

B, N, CI, CO, H, D = 8, 1024, 128, 256, 8, 32
P = 128
MT = N // P  # 8 token chunks
EPS = 1e-5
SCALE = float(1.0 / np.sqrt(np.float32(D)))
NCORES = 8
MAGIC_P1 = 0x5F3759DF + 1  # quake rsqrt magic + 1 (for the ~t + (M+1) form)

# Schraudolph exp in bf16-space (truncation-mode constant: fp32->i16
# conversion truncates).
SCHR_A = float(SCALE * 128.0 / np.log(2.0))
SCHR_B = 16249.0
EA = 512          # exp column split: ScalarE [0,EA), VectorE [EA,1024)
PIPE = 2          # attention software-pipeline depth (iterations)
WARM_MMS = 12     # dummy matmuls at start to trip the PE HAM clock gate

# bf16 blob layout (columns)
BOFF = {}
_off = 0
for _name, _w in [("w2", 512), ("wq", 512), ("wk", 512), ("wv", 512),
                  ("wo", 512), ("dg1", 512), ("g2bc2", 512), ("be2bc2", 512),
                  ("bb2row", 256)]:
    BOFF[_name] = (_off, _off + _w)
    _off += _w
BF_BLOB_W = _off
# fp32 blob layout: b2bc | bvbc | bqpp
FOFF = {"b2bc": (0, 512), "bvbc": (512, 768), "bqpp": (768, 770)}
F32_BLOB_W = 770


def _rsqrt_dve(nc, pool, var_ap, out_ap, n, tag):
    """out = 1/sqrt(var + eps) on VectorE only, batched over [128, n].

    Quake bit-trick seed + 2 Newton iterations (~5e-6 rel err). Keeps
    ScalarE free of Ln/Sqrt so its activation table never switches.
    """
    xe = pool.tile([P, n], F32, tag=f"rs_xe{tag}")
    nc.vector.tensor_scalar_add(xe, var_ap, EPS)
    y = pool.tile([P, n], F32, tag=f"rs_y{tag}")
    ti = pool.tile([P, n], I32, tag=f"rs_ti{tag}")
    # ~(x >> 1)
    nc.vector.tensor_scalar(
        out=ti, in0=xe.bitcast(I32), scalar1=1, scalar2=-1,
        op0=ALU.logical_shift_right, op1=ALU.bitwise_xor)
    # + (MAGIC+1)  ==  MAGIC - (x >> 1)
    nc.vector.tensor_scalar(
        out=y.bitcast(I32), in0=ti, scalar1=MAGIC_P1, scalar2=None,
        op0=ALU.add)
    h = pool.tile([P, n], F32, tag=f"rs_h{tag}")
    nc.vector.tensor_scalar_mul(h, xe, 0.5)
    t2 = pool.tile([P, n], F32, tag=f"rs_t2{tag}")
    for _ in range(1):
        nc.vector.tensor_mul(t2, y, y)
        nc.vector.tensor_mul(t2, t2, h)
        nc.vector.tensor_scalar(
            out=t2, in0=t2, scalar1=-1.0, scalar2=1.5, op0=ALU.mult, op1=ALU.add)
        nc.vector.tensor_mul(y, y, t2)
    nc.vector.tensor_copy(out_ap, y)


def build_bass():
    nc = bacc.Bacc()

    # per-core inputs (host pre-transposed, bf16)
    xT_d = nc.dram_tensor("xT", (CI, N), BF16, kind="ExternalInput")
    adjT_d = nc.dram_tensor("adjT", (N, N), BF16, kind="ExternalInput")
    # shared weights, host-folded and packed into two blobs
    wb_d = nc.dram_tensor("wblob", (P, BF_BLOB_W), BF16, kind="ExternalInput")
    fb_d = nc.dram_tensor("fblob", (P, F32_BLOB_W), F32, kind="ExternalInput")
    out_d = nc.dram_tensor("out", (N, CO), BF16, kind="ExternalOutput")

    with tile.TileContext(nc) as tc:
        from contextlib import ExitStack

        with ExitStack() as ctx:
            singles = ctx.enter_context(tc.tile_pool(name="singles", bufs=1))
            stemp = ctx.enter_context(tc.tile_pool(name="stemp", bufs=3))
            expT_pool = ctx.enter_context(tc.tile_pool(name="expT", bufs=6))

            # ---------------- Phase -1: input DMAs --------------------------
            # adj^T half 0 first (longest pole for the GCN), then the
            # phase-1 inputs (xT, w2, b2bc), then the rest.
            adjT_sb = singles.tile([P, MT, N], BF16)
            adjT_r = adjT_d[:].rearrange("(kc p) m -> p kc m", p=P)
            nc.scalar.dma_start(adjT_sb[:, :, 0:512], adjT_r[:, :, 0:512])
            xT_sb = singles.tile([P, N], BF16)
            nc.scalar.dma_start(xT_sb, xT_d[:])
            wb_sb = singles.tile([P, BF_BLOB_W], BF16)
            fb_sb = singles.tile([P, F32_BLOB_W], F32)
            nc.sync.dma_start(fb_sb[:, 0:512], fb_d[:][:, 0:512])
            nc.sync.dma_start(wb_sb[:, 0:512], wb_d[:][:, 0:512])
            nc.sync.dma_start(adjT_sb[:, :, 512:N], adjT_r[:, :, 512:N])
            nc.sync.dma_start(wb_sb[:, 512:BF_BLOB_W],
                              wb_d[:][:, 512:BF_BLOB_W])
            nc.sync.dma_start(fb_sb[:, 512:F32_BLOB_W],
                              fb_d[:][:, 512:F32_BLOB_W])

            def wslice(name):  # bf16 blob slice
                lo, hi = BOFF[name]
                return wb_sb[:, lo:hi]

            def fslice(name):  # fp32 blob slice
                lo, hi = FOFF[name]
                return fb_sb[:, lo:hi]

            w2_sb = wslice("w2")
            b2bc_sb = fslice("b2bc")
            bvbc_sb = fslice("bvbc")
            bqpp_sb = fslice("bqpp")
            g2bc2_sb = wslice("g2bc2")
            be2bc2_sb = wslice("be2bc2")
            bb2row_sb = wb_sb[0:1, BOFF["bb2row"][0]:BOFF["bb2row"][1]]

            def wchunk(name, kc, csl=slice(0, CO)):
                lo, _ = BOFF[name]
                base = lo + kc * CO
                return wb_sb[:, base + csl.start:base + csl.stop]

            # ---------------- Phase 0: constants + PE warmup ----------------
            ident_sb = singles.tile([P, P], BF16)
            make_identity(nc, ident_sb)
            onesrow_sb = singles.tile([1, P], BF16)
            nc.vector.memset(onesrow_sb, 1.0)
            ones_vD = singles.tile([P, D], BF16)
            nc.vector.memset(ones_vD, 1.0)
            zeros_vD = singles.tile([P, D], BF16)
            nc.vector.memset(zeros_vD, 0.0)
            # warm the ACT Exp table before attention needs it
            warm_sb = singles.tile([P, 1], F32)
            nc.vector.memset(warm_sb, 0.0)
            nc.scalar.activation(warm_sb, warm_sb, AF.Exp)

            # persistent activations
            ti_sb = singles.tile([P, MT, 2 * CO], BF16)  # [t | id] per chunk
            s_sb = singles.tile([P, MT, CO], BF16)       # pre-LN1 residual
            mv_all = singles.tile([P, MT, 2], F32)       # LN1 mean/var
            rstd_all = singles.tile([P, MT], F32)        # LN1 rstd
            mrs_all = singles.tile([P, MT], F32)         # LN1 -mu*rstd
            hT_sb = singles.tile([P, 2, N], BF16)        # hhat^T   [c, tok]
            qT_sb = singles.tile([P, 2, N], BF16)        # q^T      [c, tok]
            kT_sb = singles.tile([P, 2, N], BF16)        # k^T      [c, tok]
            v_sb = singles.tile([P, MT, CO], BF16)       # v        [tok, c]
            outT_sb = singles.tile([P, 2, N], BF16)      # attn-out^T [c, tok]
            mv2_all = singles.tile([P, MT, 2], F32)      # LN2 mean/var
            rstd2_all = singles.tile([P, MT], F32)
            mrs2_all = singles.tile([P, MT], F32)
            y_all = singles.tile([P, MT, CO], BF16)      # output staging

            with ExitStack() as pre:
                mm_ps = pre.enter_context(
                    tc.tile_pool(name="mm_ps", bufs=2, space="PSUM"))
                htr_ps = pre.enter_context(
                    tc.tile_pool(name="htr_ps", bufs=2, space="PSUM"))
                qkv_ps = pre.enter_context(
                    tc.tile_pool(name="qkv_ps", bufs=2, space="PSUM"))

                # PE HAM warmup: back-to-back dummy matmuls gated on the
                # xT DMA, so the burst bridges straight into phase 1 and the
                # GCN runs at 2.4 GHz instead of decaying during a DMA wait.
                wps = mm_ps.tile([P, 2 * CO], F32, tag="mm512")
                for _ in range(WARM_MMS):
                    nc.tensor.matmul(wps[:, 0:P], xT_sb[:, 0:P], ident_sb,
                                     start=True, stop=True,
                                     skip_group_check=True)

                # ---------------- Phase 1: [t | id] = x @ [W_g | W_it] + b --
                for m in range(MT):
                    tp = mm_ps.tile([P, 2 * CO], F32, tag="mm512")
                    nc.tensor.matmul(tp, xT_sb[:, m * P:(m + 1) * P], w2_sb,
                                     start=True, stop=True)
                    nc.vector.tensor_add(ti_sb[:, m, :], tp, b2bc_sb)

                # ---------------- Phase 2: gcn + LN1 stats per chunk --------
                # Two m-chunks accumulate in parallel chains so each
                # LDWEIGHTS overlaps the other chain's matmul.
                for m0 in range(0, MT, 2):
                    gps = [mm_ps.tile([P, CO], F32, tag="mm256",
                                      name=f"gp{i}") for i in range(2)]
                    for kc in range(MT):
                        for i in range(2):
                            m = m0 + i
                            nc.tensor.matmul(
                                gps[i], adjT_sb[:, kc, m * P:(m + 1) * P],
                                ti_sb[:, kc, 0:CO],
                                start=(kc == 0), stop=(kc == MT - 1))
                    for i in range(2):
                        m = m0 + i
                        # s = id + relu(gcn)
                        nc.vector.scalar_tensor_tensor(
                            out=s_sb[:, m, :], in0=gps[i], scalar=0.0,
                            in1=ti_sb[:, m, CO:2 * CO],
                            op0=ALU.max, op1=ALU.add)
                        stats = stemp.tile([P, 6], F32, tag="ln_stats")
                        nc.vector.bn_stats(out=stats, in_=s_sb[:, m, :])
                        nc.vector.bn_aggr(out=mv_all[:, m, :], in_=stats)

                # ------- Phase 3+4: LN1 + h^T + k/q/v, pipelined by half ----
                # Each token-half's LN1 normalize, transposes and k/q/v
                # projections are emitted as soon as that half's GCN stats
                # exist, so the VectorE rsqrt/normalize chain for half 0
                # overlaps the GCN matmuls of half 1 and the PE never
                # idles at the phase boundary.
                def ln1_half(hf):
                    sl4 = slice(4 * hf, 4 * hf + 4)
                    _rsqrt_dve(nc, stemp, mv_all[:, sl4, 1],
                               rstd_all[:, sl4], 4, f"a{hf}")
                    for m in range(4 * hf, 4 * hf + 4):
                        htmp = stemp.tile([P, CO], BF16, tag="htmp")
                        nc.vector.tensor_scalar(
                            out=htmp, in0=s_sb[:, m, :],
                            scalar1=mv_all[:, m, 0:1],
                            scalar2=rstd_all[:, m:m + 1],
                            op0=ALU.subtract, op1=ALU.mult)
                        ps = htr_ps.tile([P, 2, P], BF16, tag="htr")
                        nc.tensor.transpose(ps[:, 0, :], htmp[:, 0:P], ident_sb)
                        nc.tensor.transpose(ps[:, 1, :], htmp[:, P:CO], ident_sb)
                        nc.scalar.copy(hT_sb[:, 0, m * P:(m + 1) * P],
                                       ps[:, 0, :])
                        nc.vector.tensor_copy(hT_sb[:, 1, m * P:(m + 1) * P],
                                              ps[:, 1, :])

                def kqv_half(hf):
                    # k^T and q^T for this token-half (keys/queries
                    # 512*hf..512*hf+511 need only hT chunks 4hf..4hf+3),
                    # then v for those chunks.  k-bias dropped (softmax
                    # shift-invariant).
                    qsl = slice(hf * 512, (hf + 1) * 512)
                    for oc in range(2):
                        kp = qkv_ps.tile([P, 512], F32, tag="qkv")
                        for kc in range(2):
                            nc.tensor.matmul(
                                kp, wchunk("wk", kc, slice(oc * P, (oc + 1) * P)),
                                hT_sb[:, kc, qsl],
                                start=(kc == 0), stop=(kc == 1))
                        nc.scalar.copy(kT_sb[:, oc, qsl], kp)
                    for oc in range(2):
                        qp = qkv_ps.tile([P, 512], F32, tag="qkv")
                        for kc in range(2):
                            nc.tensor.matmul(
                                qp, wchunk("wq", kc, slice(oc * P, (oc + 1) * P)),
                                hT_sb[:, kc, qsl],
                                start=(kc == 0), stop=(kc == 1))
                        nc.scalar.activation(
                            qT_sb[:, oc, qsl], qp, AF.Identity,
                            bias=bqpp_sb[:, oc:oc + 1])
                    for m in range(4 * hf, 4 * hf + 4):
                        vp = mm_ps.tile([P, CO], F32, tag="mm256")
                        for kc in range(2):
                            nc.tensor.matmul(vp, hT_sb[:, kc, m * P:(m + 1) * P],
                                             wchunk("wv", kc),
                                             start=(kc == 0), stop=(kc == 1))
                        nc.vector.tensor_add(v_sb[:, m, :], vp, bvbc_sb)

                ln1_half(0)
                kqv_half(0)
                ln1_half(1)
                kqv_half(1)

            # ---------------- Phase 5: attention ----------------------------
            # One flat software pipeline over all 4 (token-half, head-group)
            # groups x 8 key-chunks: attn@V / denominator matmuls lag the
            # score matmuls by PIPE iterations ACROSS group boundaries, so
            # the PE never drains between groups.  Each group's softmax
            # normalization fires as soon as its last attn matmul issues.
            with ExitStack() as att:
                sc_ps = att.enter_context(
                    tc.tile_pool(name="sc_ps", bufs=3, space="PSUM"))
                acc_ps = att.enter_context(
                    tc.tile_pool(name="acc_ps", bufs=1, space="PSUM"))

                def issue_attn(qh, g, k, exs, outb, denb):
                    """attn@V + denominator MMs for (group, iteration k),
                    interleaved so adjacent matmuls hit different PSUM
                    tensors AND different column groups.  On the last
                    k-chunk, normalize the group's output."""
                    for tp in range(2):
                        for j2 in range(2):
                            hh = 4 * g + 2 * tp + j2
                            hs = 4 * g + 2 * tp + (1 - j2)
                            cp = 32 * (hh % 4)
                            cps = 32 * (hs % 4)
                            esl = slice(j2 * 512, (j2 + 1) * 512)
                            esls = slice((1 - j2) * 512, (2 - j2) * 512)
                            nc.tensor.matmul(
                                outb[cp:cp + 32, :],
                                v_sb[:, k, hh * D:(hh + 1) * D],
                                exs[tp][:, esl],
                                start=(k == 0), stop=(k == MT - 1),
                                tile_position=(0, cp),
                                skip_group_check=True)
                            nc.tensor.matmul(
                                denb[cps:cps + 32, :],
                                ones_vD,
                                exs[tp][:, esls],
                                start=(k == 0), stop=(k == MT - 1),
                                tile_position=(0, cps),
                                skip_group_check=True)
                    if k == MT - 1:
                        qsl = slice(qh * 512, (qh + 1) * 512)
                        rec = stemp.tile([P, 512], F32, tag="rec")
                        nc.vector.reciprocal_approx_fast(out=rec, in_=denb)
                        nc.vector.tensor_mul(outT_sb[:, g, qsl], outb, rec)

                pend = []
                for qh in range(2):
                    for g in range(2):
                        outb = acc_ps.tile([P, 512], F32, tag="outb",
                                           name=f"outb{qh}{g}")
                        denb = acc_ps.tile([P, 512], F32, tag="denb",
                                           name=f"denb{qh}{g}")
                        qsl = slice(qh * 512, (qh + 1) * 512)
                        for k in range(MT):
                            # ScalarE takes a wider exp share on the last
                            # two k-chunks so the VectorE queue is drained
                            # when this group's reciprocal + normalize land.
                            ea = 704 if k >= MT - 2 else EA
                            if len(pend) >= PIPE:
                                issue_attn(*pend.pop(0))
                            # all 4 score MMs back-to-back: they sit in 4
                            # distinct 32-row bands of the PE and run
                            # concurrently as one array fill.
                            scs = [sc_ps.tile([P, 1024], F32, tag="sc",
                                              name=f"sc{tp}")
                                   for tp in range(2)]
                            for tp in range(2):
                                for j2 in range(2):
                                    hh = 4 * g + 2 * tp + j2   # global head
                                    bp = 32 * (hh % 4)
                                    nc.tensor.matmul(
                                        scs[tp][:, j2 * 512:(j2 + 1) * 512],
                                        kT_sb[bp:bp + 32, g, k * P:(k + 1) * P],
                                        qT_sb[bp:bp + 32, g, qsl],
                                        start=True, stop=True,
                                        tile_position=(bp, 0))
                            exs = []
                            for tp in range(2):
                                # column-split exp: ScalarE exact [0,EA),
                                # VectorE Schraudolph [EA,1024).
                                exi = expT_pool.tile([P, 1024], I16, tag="ex")
                                exb = exi.bitcast(BF16)
                                nc.scalar.activation(
                                    exb[:, 0:ea], scs[tp][:, 0:ea], AF.Exp,
                                    scale=SCALE)
                                nc.vector.tensor_scalar(
                                    out=exi[:, ea:1024],
                                    in0=scs[tp][:, ea:1024],
                                    scalar1=SCHR_A, scalar2=SCHR_B,
                                    op0=ALU.mult, op1=ALU.add)
                                exs.append(exb)
                            pend.append((qh, g, k, exs, outb, denb))
                for item in pend:
                    issue_attn(*item)

            # ---------------- Phase 6: projection + LN2 + store -------------
            # Pipelined per pair of token chunks: matmuls -> stats -> rsqrt
            # -> ScalarE normalize -> g2/beta2 -> DMA.
            with ExitStack() as post:
                proj_ps = post.enter_context(
                    tc.tile_pool(name="proj_ps", bufs=4, space="PSUM"))

                for pair in range(4):
                    m0 = 2 * pair
                    pp2 = proj_ps.tile([P, 2, CO], F32, tag="proj")
                    for i in range(2):
                        m = m0 + i
                        pp = pp2[:, i, :]
                        # s2 = (beta1+b_o) + out@W_o + hhat*g1, all in PSUM
                        nc.tensor.matmul(pp, onesrow_sb, bb2row_sb,
                                         start=True, stop=False)
                        for cc in range(2):
                            nc.tensor.matmul(
                                pp, outT_sb[:, cc, m * P:(m + 1) * P],
                                wchunk("wo", cc), start=False, stop=False)
                        for cc in range(2):
                            nc.tensor.matmul(
                                pp, hT_sb[:, cc, m * P:(m + 1) * P],
                                wchunk("dg1", cc), start=False, stop=(cc == 1))
                        stats = stemp.tile([P, 6], F32, tag="ln_stats")
                        nc.vector.bn_stats(out=stats, in_=pp)
                        nc.vector.bn_aggr(out=mv2_all[:, m, :], in_=stats)
                    psl = slice(m0, m0 + 2)
                    _rsqrt_dve(nc, stemp, mv2_all[:, psl, 1],
                               rstd2_all[:, psl], 2, f"b{pair}")
                    nc.vector.scalar_tensor_tensor(
                        out=mrs2_all[:, psl], in0=mv2_all[:, psl, 0],
                        scalar=-1.0, in1=rstd2_all[:, psl],
                        op0=ALU.mult, op1=ALU.mult)
                    for i in range(2):
                        m = m0 + i
                        nc.scalar.activation(
                            y_all[:, m, :], pp2[:, i, :], AF.Identity,
                            bias=mrs2_all[:, m:m + 1],
                            scale=rstd2_all[:, m:m + 1])
                    yp = y_all[:, m0:m0 + 2, :]
                    nc.vector.tensor_mul(yp, yp, g2bc2_sb)
                    nc.vector.tensor_add(yp, yp, be2bc2_sb)
                    nc.sync.dma_start(
                        out_d[:].rearrange("(mt p) c -> p mt c", p=P)
                        [:, psl, :], yp)

    nc.finalize()
    return nc


_CACHE = {}


def _get_nc():
    if "nc" not in _CACHE:
        _CACHE["nc"] = build_bass()
    return _CACHE["nc"]


def _prep_shared(inputs):
    """Host-side weight prep: fold LN1 gamma/beta, pre-transpose, cast,
    pack into two blobs."""
    import ml_dtypes
    bf16 = ml_dtypes.bfloat16
    f32 = np.float32

    W_it = np.asarray(inputs["W_it"], f32)
    b_it = np.asarray(inputs["b_it"], f32)
    W_g = np.asarray(inputs["W_g"], f32)
    b_g = np.asarray(inputs["b_g"], f32)
    W_q = np.asarray(inputs["W_q"], f32)
    b_q = np.asarray(inputs["b_q"], f32)
    W_k = np.asarray(inputs["W_k"], f32)
    W_v = np.asarray(inputs["W_v"], f32)
    b_v = np.asarray(inputs["b_v"], f32)
    W_o = np.asarray(inputs["W_o"], f32)
    b_o = np.asarray(inputs["b_o"], f32)
    g1 = np.asarray(inputs["g1"], f32)
    beta1 = np.asarray(inputs["beta1"], f32)
    g2 = np.asarray(inputs["g2"], f32)
    beta2 = np.asarray(inputs["beta2"], f32)

    def chunk2(w):  # [CO, CO] -> [P, 512] as (kc p) n -> p (kc n)
        return w.reshape(2, P, CO).transpose(1, 0, 2).reshape(P, 2 * CO)

    wblob = np.zeros((P, BF_BLOB_W), f32)

    def put(name, arr):
        lo, hi = BOFF[name]
        wblob[:, lo:hi] = arr

    put("w2", np.concatenate([W_g, W_it], axis=1))
    put("wq", chunk2(g1[:, None] * W_q))
    put("wk", chunk2(g1[:, None] * W_k))
    put("wv", chunk2(g1[:, None] * W_v))
    put("wo", chunk2(W_o))
    put("dg1", chunk2(np.diag(g1)))
    put("g2bc2", np.tile(g2[None, :], (P, 2)))
    put("be2bc2", np.tile(beta2[None, :], (P, 2)))
    bb2 = np.zeros((P, CO), f32)
    bb2[0] = beta1 + b_o
    put("bb2row", bb2)

    fblob = np.zeros((P, F32_BLOB_W), f32)
    fblob[:, 0:512] = np.concatenate([b_g, b_it])[None, :]
    fblob[:, 512:768] = (b_v + beta1 @ W_v)[None, :]
    fblob[:, 768:770] = (b_q + beta1 @ W_q).reshape(2, P).T

    return {
        "wblob": np.ascontiguousarray(wblob).astype(bf16),
        "fblob": np.ascontiguousarray(fblob),
    }


def run(inputs, trace=False):
    import ml_dtypes
    bf16 = ml_dtypes.bfloat16
    nc = _get_nc()
    shared = _prep_shared(inputs)
    x = np.asarray(inputs["x"], np.float32)
    adj = np.asarray(inputs["adj"], np.float32)
    in_maps = []
    for b in range(NCORES):
        m = dict(shared)
        m["xT"] = np.ascontiguousarray(x[b].T).astype(bf16)
        m["adjT"] = np.ascontiguousarray(adj[b].T).astype(bf16)
        in_maps.append(m)
    res = run_bass_kernel_spmd(nc, in_maps, core_ids=list(range(NCORES)),
                               trace=trace)
    out = np.stack(
        [res.results[b]["out"].astype(np.float32) for b in range(NCORES)],
        axis=0)
    return out, res


def kernel(**inputs):
    out, _ = run(inputs, trace=False)
    return out


# revision 30
# speedup vs baseline: 1.0040x; 1.0040x over previous
"""AttentionGCNLayer Trainium2 kernel.

Per-sample computation (B=8 samples -> 8 NeuronCores, data-parallel):
  identity = x @ W_it + b_it
  gcn      = relu(adj @ (x @ W_g + b_g))
  h        = LN1(identity + gcn)
  attn     = MHSA(h)  (8 heads, D=32)
  out      = LN2(h + attn)

Design notes (~2.1x faster than the previous kernel, ~117us/core):
  - All layout work on the host: x^T and adj^T uploaded pre-transposed in
    bf16, LN1 gamma/beta folded into W_q/W_k/W_v, broadcast bias tiles
    pre-replicated, shared weights packed into two DMA blobs ordered so
    phase-1 inputs and the first adj^T half land first.
  - Every matmul is bf16 (fp32 runs at 4 cycles/row on the PE; bf16 at 1).
  - Softmax exp of the 8.4M-element score matrix is the dominant
    elementwise cost (a PSUM->SBUF crossing at ~4B/lane/cycle, ~65us of
    engine time).  Each score tile is split column-wise: ScalarE does
    exact exp on cols [0,EA), VectorE does Schraudolph bit-trick exp
    (i16 = trunc(s*SCALE*128/ln2 + 16249); bitcast i16 -> bf16) on
    [EA,1024) -- both engines run concurrently on the same tile.
    End-to-end error of the trick is negligible (3.41e-3 vs 3.41e-3
    exact; tolerance 2e-2).
  - The attention runs as ONE flat software pipeline over all 4
    (token-half x head-group) groups x 8 key-chunks: attn@V/denominator
    matmuls lag the score matmuls by PIPE iterations ACROSS group
    boundaries, each group's softmax normalization fires when its last
    attn matmul issues, and all 4 score matmuls of an iteration are
    emitted back-to-back (4 distinct 32-row PE bands -> one concurrent
    array fill).  The score PSUM pool keeps 3 tile generations so the
    exp engines stay off the PSUM-recycle critical cycle.
  - A short burst of dummy matmuls gated on the xT DMA trips the PE HAM
    activity window so phase 1 / GCN run at 2.4 GHz instead of 1.2.
  - k-projection bias dropped: softmax is invariant to score shifts
    constant along the key axis.
  - LN2 residual (hhat*g1) folded into the output projection as diag(g1)
    matmuls; (beta1+b_o) preloaded into PSUM via a contraction-1 ones
    matmul; LN rsqrt batched on VectorE (quake seed + 1 Newton step);
    LN2 normalize on ScalarE via Identity(scale=rstd, bias=-mu*rstd);
    ScalarE takes a wider exp share (704 cols) on each group's last two
    k-chunks so VectorE is drained when recip+normalize land.
  - Softmax denominators via ones-matmul (partition-direction sum on
    PE), scoresT layout [key on partition, query on free].
  - Output returned bf16 and cast to fp32 on the host.
"""

import sys

sys.path.insert(0, "/opt/trn_rl_repo")

import numpy as np

import concourse.bass as bass
import concourse.tile as tile
from concourse import bacc, mybir
from concourse.bass_utils import run_bass_kernel_spmd
from concourse.masks import make_identity

F32 = mybir.dt.float32
BF16 = mybir.dt.bfloat16
I16 = mybir.dt.int16
I32 = mybir.dt.int32
AF = mybir.ActivationFunctionType
ALU = mybir.AluOpType

B, N, CI, CO, H, D = 8, 1024, 128, 256, 8, 32
P = 128
MT = N // P  # 8 token chunks
EPS = 1e-5
SCALE = float(1.0 / np.sqrt(np.float32(D)))
NCORES = 8
MAGIC_P1 = 0x5F3759DF + 1  # quake rsqrt magic + 1 (for the ~t + (M+1) form)

# Schraudolph exp in bf16-space (truncation-mode constant: fp32->i16
# conversion truncates).
SCHR_A = float(SCALE * 128.0 / np.log(2.0))
SCHR_B = 16249.0
EA = 512          # exp column split: ScalarE [0,EA), VectorE [EA,1024)
PIPE = 2          # attention software-pipeline depth (iterations)
WARM_MMS = 12     # dummy matmuls at start to trip the PE HAM clock gate

# bf16 blob layout (columns)
BOFF = {}
_off = 0
for _name, _w in [("w2", 512), ("wq", 512), ("wk", 512), ("wv", 512),
                  ("wo", 512), ("dg1", 512), ("g2bc2", 512), ("be2bc2", 512),
                  ("bb2row", 256)]:
    BOFF[_name] = (_off, _off + _w)
    _off += _w
BF_BLOB_W = _off
# fp32 blob layout: b2bc | bvbc | bqpp
FOFF = {"b2bc": (0, 512), "bvbc": (512, 768), "bqpp": (768, 770)}
F32_BLOB_W = 770


def _rsqrt_dve(nc, pool, var_ap, out_ap, n, tag):
    """out = 1/sqrt(var + eps) on VectorE only, batched over [128, n].

    Quake bit-trick seed + 1 Newton iteration (~0.17% rel err -- invisible
    against the 2e-2 gate). Keeps ScalarE free of Ln/Sqrt so its
    activation table never switches.
    """
    xe = pool.tile([P, n], F32, tag=f"rs_xe{tag}")
    nc.vector.tensor_scalar_add(xe, var_ap, EPS)
    y = pool.tile([P, n], F32, tag=f"rs_y{tag}")
    ti = pool.tile([P, n], I32, tag=f"rs_ti{tag}")
    # ~(x >> 1)
    nc.vector.tensor_scalar(
        out=ti, in0=xe.bitcast(I32), scalar1=1, scalar2=-1,
        op0=ALU.logical_shift_right, op1=ALU.bitwise_xor)
    # + (MAGIC+1)  ==  MAGIC - (x >> 1)
    nc.vector.tensor_scalar(
        out=y.bitcast(I32), in0=ti, scalar1=MAGIC_P1, scalar2=None,
        op0=ALU.add)
    h = pool.tile([P, n], F32, tag=f"rs_h{tag}")
    nc.vector.tensor_scalar_mul(h, xe, 0.5)
    t2 = pool.tile([P, n], F32, tag=f"rs_t2{tag}")
    for _ in range(1):
        nc.vector.tensor_mul(t2, y, y)
        nc.vector.tensor_mul(t2, t2, h)
        nc.vector.tensor_scalar(
            out=t2, in0=t2, scalar1=-1.0, scalar2=1.5, op0=ALU.mult, op1=ALU.add)
        nc.vector.tensor_mul(y, y, t2)
    nc.vector.tensor_copy(out_ap, y)


def build_bass():
    nc = bacc.Bacc()

    # per-core inputs (host pre-transposed, bf16)
    xT_d = nc.dram_tensor("xT", (CI, N), BF16, kind="ExternalInput")
    adjT_d = nc.dram_tensor("adjT", (N, N), BF16, kind="ExternalInput")
    # shared weights, host-folded and packed into two blobs
    wb_d = nc.dram_tensor("wblob", (P, BF_BLOB_W), BF16, kind="ExternalInput")
    fb_d = nc.dram_tensor("fblob", (P, F32_BLOB_W), F32, kind="ExternalInput")
    out_d = nc.dram_tensor("out", (N, CO), BF16, kind="ExternalOutput")

    with tile.TileContext(nc) as tc:
        from contextlib import ExitStack

        with ExitStack() as ctx:
            singles = ctx.enter_context(tc.tile_pool(name="singles", bufs=1))
            stemp = ctx.enter_context(tc.tile_pool(name="stemp", bufs=3))
            expT_pool = ctx.enter_context(tc.tile_pool(name="expT", bufs=6))

            # ---------------- Phase -1: input DMAs --------------------------
            # adj^T half 0 first (longest pole for the GCN), then the
            # phase-1 inputs (xT, w2, b2bc), then the rest.
            adjT_sb = singles.tile([P, MT, N], BF16)
            adjT_r = adjT_d[:].rearrange("(kc p) m -> p kc m", p=P)
            nc.scalar.dma_start(adjT_sb[:, :, 0:512], adjT_r[:, :, 0:512])
            xT_sb = singles.tile([P, N], BF16)
            nc.scalar.dma_start(xT_sb, xT_d[:])
            wb_sb = singles.tile([P, BF_BLOB_W], BF16)
            fb_sb = singles.tile([P, F32_BLOB_W], F32)
            nc.sync.dma_start(fb_sb[:, 0:512], fb_d[:][:, 0:512])
            nc.sync.dma_start(wb_sb[:, 0:512], wb_d[:][:, 0:512])
            nc.sync.dma_start(adjT_sb[:, :, 512:N], adjT_r[:, :, 512:N])
            nc.sync.dma_start(wb_sb[:, 512:BF_BLOB_W],
                              wb_d[:][:, 512:BF_BLOB_W])
            nc.sync.dma_start(fb_sb[:, 512:F32_BLOB_W],
                              fb_d[:][:, 512:F32_BLOB_W])

            def wslice(name):  # bf16 blob slice
                lo, hi = BOFF[name]
                return wb_sb[:, lo:hi]

            def fslice(name):  # fp32 blob slice
                lo, hi = FOFF[name]
                return fb_sb[:, lo:hi]

            w2_sb = wslice("w2")
            b2bc_sb = fslice("b2bc")
            bvbc_sb = fslice("bvbc")
            bqpp_sb = fslice("bqpp")
            g2bc2_sb = wslice("g2bc2")
            be2bc2_sb = wslice("be2bc2")
            bb2row_sb = wb_sb[0:1, BOFF["bb2row"][0]:BOFF["bb2row"][1]]

            def wchunk(name, kc, csl=slice(0, CO)):
                lo, _ = BOFF[name]
                base = lo + kc * CO
                return wb_sb[:, base + csl.start:base + csl.stop]

            # ---------------- Phase 0: constants + PE warmup ----------------
            ident_sb = singles.tile([P, P], BF16)
            make_identity(nc, ident_sb)
            onesrow_sb = singles.tile([1, P], BF16)
            nc.vector.memset(onesrow_sb, 1.0)
            ones_vD = singles.tile([P, D], BF16)
            nc.vector.memset(ones_vD, 1.0)
            zeros_vD = singles.tile([P, D], BF16)
            nc.vector.memset(zeros_vD, 0.0)
            # warm the ACT Exp table before attention needs it
            warm_sb = singles.tile([P, 1], F32)
            nc.vector.memset(warm_sb, 0.0)
            nc.scalar.activation(warm_sb, warm_sb, AF.Exp)

            # persistent activations
            ti_sb = singles.tile([P, MT, 2 * CO], BF16)  # [t | id] per chunk
            s_sb = singles.tile([P, MT, CO], BF16)       # pre-LN1 residual
            mv_all = singles.tile([P, MT, 2], F32)       # LN1 mean/var
            rstd_all = singles.tile([P, MT], F32)        # LN1 rstd
            mrs_all = singles.tile([P, MT], F32)         # LN1 -mu*rstd
            hT_sb = singles.tile([P, 2, N], BF16)        # hhat^T   [c, tok]
            qT_sb = singles.tile([P, 2, N], BF16)        # q^T      [c, tok]
            kT_sb = singles.tile([P, 2, N], BF16)        # k^T      [c, tok]
            v_sb = singles.tile([P, MT, CO], BF16)       # v        [tok, c]
            outT_sb = singles.tile([P, 2, N], BF16)      # attn-out^T [c, tok]
            mv2_all = singles.tile([P, MT, 2], F32)      # LN2 mean/var
            rstd2_all = singles.tile([P, MT], F32)
            mrs2_all = singles.tile([P, MT], F32)
            y_all = singles.tile([P, MT, CO], BF16)      # output staging

            with ExitStack() as pre:
                mm_ps = pre.enter_context(
                    tc.tile_pool(name="mm_ps", bufs=2, space="PSUM"))
                htr_ps = pre.enter_context(
                    tc.tile_pool(name="htr_ps", bufs=2, space="PSUM"))
                qkv_ps = pre.enter_context(
                    tc.tile_pool(name="qkv_ps", bufs=2, space="PSUM"))

                # PE HAM warmup: back-to-back dummy matmuls gated on the
                # xT DMA, so the burst bridges straight into phase 1 and the
                # GCN runs at 2.4 GHz instead of decaying during a DMA wait.
                wps = mm_ps.tile([P, 2 * CO], F32, tag="mm512")
                for _ in range(WARM_MMS):
                    nc.tensor.matmul(wps[:, 0:P], xT_sb[:, 0:P], ident_sb,
                                     start=True, stop=True,
                                     skip_group_check=True)

                # ---------------- Phase 1: [t | id] = x @ [W_g | W_it] + b --
                for m in range(MT):
                    tp = mm_ps.tile([P, 2 * CO], F32, tag="mm512")
                    nc.tensor.matmul(tp, xT_sb[:, m * P:(m + 1) * P], w2_sb,
                                     start=True, stop=True)
                    nc.vector.tensor_add(ti_sb[:, m, :], tp, b2bc_sb)

                # ---------------- Phase 2: gcn + LN1 stats per chunk --------
                # Two m-chunks accumulate in parallel chains so each
                # LDWEIGHTS overlaps the other chain's matmul.
                for m0 in range(0, MT, 2):
                    gps = [mm_ps.tile([P, CO], F32, tag="mm256",
                                      name=f"gp{i}") for i in range(2)]
                    for kc in range(MT):
                        for i in range(2):
                            m = m0 + i
                            nc.tensor.matmul(
                                gps[i], adjT_sb[:, kc, m * P:(m + 1) * P],
                                ti_sb[:, kc, 0:CO],
                                start=(kc == 0), stop=(kc == MT - 1))
                    for i in range(2):
                        m = m0 + i
                        # s = id + relu(gcn)
                        nc.vector.scalar_tensor_tensor(
                            out=s_sb[:, m, :], in0=gps[i], scalar=0.0,
                            in1=ti_sb[:, m, CO:2 * CO],
                            op0=ALU.max, op1=ALU.add)
                        stats = stemp.tile([P, 6], F32, tag="ln_stats")
                        nc.vector.bn_stats(out=stats, in_=s_sb[:, m, :])
                        nc.vector.bn_aggr(out=mv_all[:, m, :], in_=stats)

                # ------- Phase 3+4: LN1 + h^T + k/q/v, pipelined by half ----
                # Each token-half's LN1 normalize, transposes and k/q/v
                # projections are emitted as soon as that half's GCN stats
                # exist, so the VectorE rsqrt/normalize chain for half 0
                # overlaps the GCN matmuls of half 1 and the PE never
                # idles at the phase boundary.
                def ln1_half(hf):
                    sl4 = slice(4 * hf, 4 * hf + 4)
                    _rsqrt_dve(nc, stemp, mv_all[:, sl4, 1],
                               rstd_all[:, sl4], 4, f"a{hf}")
                    for m in range(4 * hf, 4 * hf + 4):
                        htmp = stemp.tile([P, CO], BF16, tag="htmp")
                        nc.vector.tensor_scalar(
                            out=htmp, in0=s_sb[:, m, :],
                            scalar1=mv_all[:, m, 0:1],
                            scalar2=rstd_all[:, m:m + 1],
                            op0=ALU.subtract, op1=ALU.mult)
                        ps = htr_ps.tile([P, 2, P], BF16, tag="htr")
                        nc.tensor.transpose(ps[:, 0, :], htmp[:, 0:P], ident_sb)
                        nc.tensor.transpose(ps[:, 1, :], htmp[:, P:CO], ident_sb)
                        nc.scalar.copy(hT_sb[:, 0, m * P:(m + 1) * P],
                                       ps[:, 0, :])
                        nc.vector.tensor_copy(hT_sb[:, 1, m * P:(m + 1) * P],
                                              ps[:, 1, :])

                def kqv_half(hf):
                    # k^T and q^T for this token-half (keys/queries
                    # 512*hf..512*hf+511 need only hT chunks 4hf..4hf+3),
                    # then v for those chunks.  k-bias dropped (softmax
                    # shift-invariant).
                    qsl = slice(hf * 512, (hf + 1) * 512)
                    for oc in range(2):
                        kp = qkv_ps.tile([P, 512], F32, tag="qkv")
                        for kc in range(2):
                            nc.tensor.matmul(
                                kp, wchunk("wk", kc, slice(oc * P, (oc + 1) * P)),
                                hT_sb[:, kc, qsl],
                                start=(kc == 0), stop=(kc == 1))
                        nc.scalar.copy(kT_sb[:, oc, qsl], kp)
                    for oc in range(2):
                        qp = qkv_ps.tile([P, 512], F32, tag="qkv")
                        for kc in range(2):
                            nc.tensor.matmul(
                                qp, wchunk("wq", kc, slice(oc * P, (oc + 1) * P)),
                                hT_sb[:, kc, qsl],
                                start=(kc == 0), stop=(kc == 1))
                        nc.scalar.activation(
                            qT_sb[:, oc, qsl], qp, AF.Identity,
                            bias=bqpp_sb[:, oc:oc + 1])
                    for m in range(4 * hf, 4 * hf + 4):
                        vp = mm_ps.tile([P, CO], F32, tag="mm256")
                        for kc in range(2):
                            nc.tensor.matmul(vp, hT_sb[:, kc, m * P:(m + 1) * P],
                                             wchunk("wv", kc),
                                             start=(kc == 0), stop=(kc == 1))
                        nc.vector.tensor_add(v_sb[:, m, :], vp, bvbc_sb)

                ln1_half(0)
                kqv_half(0)
                ln1_half(1)
                kqv_half(1)

            # ---------------- Phase 5: attention ----------------------------
            # One flat software pipeline over all 4 (token-half, head-group)
            # groups x 8 key-chunks: attn@V / denominator matmuls lag the
            # score matmuls by PIPE iterations ACROSS group boundaries, so
            # the PE never drains between groups.  Each group's softmax
            # normalization fires as soon as its last attn matmul issues.
            with ExitStack() as att:
                sc_ps = att.enter_context(
                    tc.tile_pool(name="sc_ps", bufs=3, space="PSUM"))
                acc_ps = att.enter_context(
                    tc.tile_pool(name="acc_ps", bufs=1, space="PSUM"))

                def issue_attn(qh, g, k, exs, outb, denb):
                    """attn@V + denominator MMs for (group, iteration k),
                    interleaved so adjacent matmuls hit different PSUM
                    tensors AND different column groups.  On the last
                    k-chunk, normalize the group's output."""
                    for tp in range(2):
                        for j2 in range(2):
                            hh = 4 * g + 2 * tp + j2
                            hs = 4 * g + 2 * tp + (1 - j2)
                            cp = 32 * (hh % 4)
                            cps = 32 * (hs % 4)
                            esl = slice(j2 * 512, (j2 + 1) * 512)
                            esls = slice((1 - j2) * 512, (2 - j2) * 512)
                            nc.tensor.matmul(
                                outb[cp:cp + 32, :],
                                v_sb[:, k, hh * D:(hh + 1) * D],
                                exs[tp][:, esl],
                                start=(k == 0), stop=(k == MT - 1),
                                tile_position=(0, cp),
                                skip_group_check=True)
                            nc.tensor.matmul(
                                denb[cps:cps + 32, :],
                                ones_vD,
                                exs[tp][:, esls],
                                start=(k == 0), stop=(k == MT - 1),
                                tile_position=(0, cps),
                                skip_group_check=True)
                    if k == MT - 1:
                        qsl = slice(qh * 512, (qh + 1) * 512)
                        rec = stemp.tile([P, 512], F32, tag="rec")
                        nc.vector.reciprocal_approx_fast(out=rec, in_=denb)
                        nc.vector.tensor_mul(outT_sb[:, g, qsl], outb, rec)

                pend = []
                for qh in range(2):
                    for g in range(2):
                        outb = acc_ps.tile([P, 512], F32, tag="outb",
                                           name=f"outb{qh}{g}")
                        denb = acc_ps.tile([P, 512], F32, tag="denb",
                                           name=f"denb{qh}{g}")
                        qsl = slice(qh * 512, (qh + 1) * 512)
                        for k in range(MT):
                            # ScalarE takes a wider exp share on the last
                            # two k-chunks so the VectorE queue is drained
                            # when this group's reciprocal + normalize land.
                            ea = 704 if k >= MT - 2 else EA
                            if len(pend) >= PIPE:
                                issue_attn(*pend.pop(0))
                            # all 4 score MMs back-to-back: they sit in 4
                            # distinct 32-row bands of the PE and run
                            # concurrently as one array fill.
                            scs = [sc_ps.tile([P, 1024], F32, tag="sc",
                                              name=f"sc{tp}")
                                   for tp in range(2)]
                            for tp in range(2):
                                for j2 in range(2):
                                    hh = 4 * g + 2 * tp + j2   # global head
                                    bp = 32 * (hh % 4)
                                    nc.tensor.matmul(
                                        scs[tp][:, j2 * 512:(j2 + 1) * 512],
                                        kT_sb[bp:bp + 32, g, k * P:(k + 1) * P],
                                        qT_sb[bp:bp + 32, g, qsl],
                                        start=True, stop=True,
                                        tile_position=(bp, 0))
                            exs = []
                            for tp in range(2):
                                # column-split exp: ScalarE exact [0,EA),
                                # VectorE Schraudolph [EA,1024).
                                exi = expT_pool.tile([P, 1024], I16, tag="ex")
                                exb = exi.bitcast(BF16)
                                nc.scalar.activation(
                                    exb[:, 0:ea], scs[tp][:, 0:ea], AF.Exp,
                                    scale=SCALE)
                                nc.vector.tensor_scalar(
                                    out=exi[:, ea:1024],
                                    in0=scs[tp][:, ea:1024],
                                    scalar1=SCHR_A, scalar2=SCHR_B,
                                    op0=ALU.mult, op1=ALU.add)
                                exs.append(exb)
                            pend.append((qh, g, k, exs, outb, denb))
                for item in pend:
                    issue_attn(*item)

            # ---------------- Phase 6: projection + LN2 + store -------------
            # Pipelined per pair of token chunks: matmuls -> stats -> rsqrt
            # -> ScalarE normalize -> g2/beta2 -> DMA.
            with ExitStack() as post:
                proj_ps = post.enter_context(
                    tc.tile_pool(name="proj_ps", bufs=4, space="PSUM"))

                for pair in range(4):
                    m0 = 2 * pair
                    pp2 = proj_ps.tile([P, 2, CO], F32, tag="proj")
                    for i in range(2):
                        m = m0 + i
                        pp = pp2[:, i, :]
                        # s2 = (beta1+b_o) + out@W_o + hhat*g1, all in PSUM
                        nc.tensor.matmul(pp, onesrow_sb, bb2row_sb,
                                         start=True, stop=False)
                        for cc in range(2):
                            nc.tensor.matmul(
                                pp, outT_sb[:, cc, m * P:(m + 1) * P],
                                wchunk("wo", cc), start=False, stop=False)
                        for cc in range(2):
                            nc.tensor.matmul(
                                pp, hT_sb[:, cc, m * P:(m + 1) * P],
                                wchunk("dg1", cc), start=False, stop=(cc == 1))
                        stats = stemp.tile([P, 6], F32, tag="ln_stats")
                        nc.vector.bn_stats(out=stats, in_=pp)
                        nc.vector.bn_aggr(out=mv2_all[:, m, :], in_=stats)
                    psl = slice(m0, m0 + 2)
                    _rsqrt_dve(nc, stemp, mv2_all[:, psl, 1],
                               rstd2_all[:, psl], 2, f"b{pair}")
                    nc.vector.scalar_tensor_tensor(
                        out=mrs2_all[:, psl], in0=mv2_all[:, psl, 0],
                        scalar=-1.0, in1=rstd2_all[:, psl],
                        op0=ALU.mult, op1=ALU.mult)
                    for i in range(2):
                        m = m0 + i
                        nc.scalar.activation(
                            y_all[:, m, :], pp2[:, i, :], AF.Identity,
                            bias=mrs2_all[:, m:m + 1],
                            scale=rstd2_all[:, m:m + 1])
                    yp = y_all[:, m0:m0 + 2, :]
                    nc.vector.tensor_mul(yp, yp, g2bc2_sb)
                    nc.vector.tensor_add(yp, yp, be2bc2_sb)
                    nc.sync.dma_start(
                        out_d[:].rearrange("(mt p) c -> p mt c", p=P)
                        [:, psl, :], yp)

    nc.finalize()
    return nc


_CACHE = {}


def _get_nc():
    if "nc" not in _CACHE:
        _CACHE["nc"] = build_bass()
    return _CACHE["nc"]


def _prep_shared(inputs):
    """Host-side weight prep: fold LN1 gamma/beta, pre-transpose, cast,
    pack into two blobs."""
    import ml_dtypes
    bf16 = ml_dtypes.bfloat16
    f32 = np.float32

    W_it = np.asarray(inputs["W_it"], f32)
    b_it = np.asarray(inputs["b_it"], f32)
    W_g = np.asarray(inputs["W_g"], f32)
    b_g = np.asarray(inputs["b_g"], f32)
    W_q = np.asarray(inputs["W_q"], f32)
    b_q = np.asarray(inputs["b_q"], f32)
    W_k = np.asarray(inputs["W_k"], f32)
    W_v = np.asarray(inputs["W_v"], f32)
    b_v = np.asarray(inputs["b_v"], f32)
    W_o = np.asarray(inputs["W_o"], f32)
    b_o = np.asarray(inputs["b_o"], f32)
    g1 = np.asarray(inputs["g1"], f32)
    beta1 = np.asarray(inputs["beta1"], f32)
    g2 = np.asarray(inputs["g2"], f32)
    beta2 = np.asarray(inputs["beta2"], f32)

    def chunk2(w):  # [CO, CO] -> [P, 512] as (kc p) n -> p (kc n)
        return w.reshape(2, P, CO).transpose(1, 0, 2).reshape(P, 2 * CO)

    wblob = np.zeros((P, BF_BLOB_W), f32)

    def put(name, arr):
        lo, hi = BOFF[name]
        wblob[:, lo:hi] = arr

    put("w2", np.concatenate([W_g, W_it], axis=1))
    put("wq", chunk2(g1[:, None] * W_q))
    put("wk", chunk2(g1[:, None] * W_k))
    put("wv", chunk2(g1[:, None] * W_v))
    put("wo", chunk2(W_o))
    put("dg1", chunk2(np.diag(g1)))
    put("g2bc2", np.tile(g2[None, :], (P, 2)))
    put("be2bc2", np.tile(beta2[None, :], (P, 2)))
    bb2 = np.zeros((P, CO), f32)
    bb2[0] = beta1 + b_o
    put("bb2row", bb2)

    fblob = np.zeros((P, F32_BLOB_W), f32)
    fblob[:, 0:512] = np.concatenate([b_g, b_it])[None, :]
    fblob[:, 512:768] = (b_v + beta1 @ W_v)[None, :]
    fblob[:, 768:770] = (b_q + beta1 @ W_q).reshape(2, P).T

    return {
        "wblob": np.ascontiguousarray(wblob).astype(bf16),
        "fblob": np.ascontiguousarray(fblob),
    }


def run(inputs, trace=False):
    import ml_dtypes
    bf16 = ml_dtypes.bfloat16
    nc = _get_nc()
    shared = _prep_shared(inputs)
    x = np.asarray(inputs["x"], np.float32)
    adj = np.asarray(inputs["adj"], np.float32)
    in_maps = []
    for b in range(NCORES):
        m = dict(shared)
        m["xT"] = np.ascontiguousarray(x[b].T).astype(bf16)
        m["adjT"] = np.ascontiguousarray(adj[b].T).astype(bf16)
        in_maps.append(m)
    res = run_bass_kernel_spmd(nc, in_maps, core_ids=list(range(NCORES)),
                               trace=trace)
    out = np.stack(
        [res.results[b]["out"].astype(np.float32) for b in range(NCORES)],
        axis=0)
    return out, res


def kernel(**inputs):
    out, _ = run(inputs, trace=False)
    return out
